# revision 27
# baseline (speedup 1.0000x reference)
"""Cross-attention (single-head, residual) Bass/Tile kernel for Trainium2.

Problem: y = x + (softmax((x' Wq + bq)(ctx Wk + bk)^T / sqrt(C)) (ctx Wv + bv)) Wo + bo
  x: [B=8, C=512, H=64, W=64], context: [B=8, Lc=512, CTX=768]

Sharding: pure data-parallel over batch - one batch element per NeuronCore,
no collectives.

Weight folding (host, exact): softmax is invariant to per-row constants, so
  sim ~ x^T (Wq Wk^T) ctx^T + (Wk bq)^T ctx^T      (x^T Wq bk and bq.bk drop)
  out = attn ctx (Wv Wo) + (Wo^T bv + bo)          (attn rows sum to 1)
Device sees A = Wq Wk^T and Wvo = Wv Wo only: phase A is two 12-matmul
passes (G = A ctx^T, vW = ctx Wvo + bvo) instead of four.

Per hw-tile the streaming loop is two fp8 DoubleRow matmul stages:
  simT[lc,hw] = sum_c x[c,hw] G[c,lc]
  eT = exp(SCALE*simT)  (one ACTIVATE per 2-bank psum pair)
  yT[hw,c] = (eT^T vW) * (1/colsum eT) + x^T   (eT stationary, colsum rides
             the same weight loads as a free-dim-1 matmul into a psum column)

All DRAM tensors are host-pre-swizzled into the exact SBUF layout so every
DMA line is contiguous per partition (large descriptors, minimal HWDGE issue
time). Loads go on the sync HWDGE ring, y stores on the scalar HWDGE ring.
No gpsimd queue (its SWDGE drain cost ~6us in the epilogue). fp8 weights are
host-scaled out of the subnormal range; evictions unscale. The colsum "ones"
are memset to the vW storage scale so normalization unscales for free.
"""

import numpy as np
import ml_dtypes

B = 8
C = 512
CTX = 768
Lc = 512
HH = 64
WW = 64
HW = HH * WW          # 4096
N_CORES = 8
P = 128
HT = 512              # hw tile (free-dim) width
N_HT = HW // HT       # 8
NCH = HT // P         # 4 hw chunks per tile
KC = C // P           # 4
KX = CTX // P         # 6
KL = Lc // P          # 4
SCALE = float(C) ** -0.5
WSA = 32.0            # host fp8 scale of A = Wq Wk^T
WSV = 64.0            # host fp8 scale of Wvo = Wv Wo
WSO = 4.0             # storage scale of vW (and the colsum ones value)
WSQ = 32.0            # host fp8 scale of Wk bq
N_WARM = 9           # PE warm-up matmuls during the initial DMA wait

NP_BF16 = ml_dtypes.bfloat16
NP_FP8 = ml_dtypes.float8_e4m3

_cache = {}


def _build_nc(with_bq=True, with_bvo=True):
    import concourse.mybir as mybir
    import concourse.bass as bass
    import concourse.tile as tile
    from concourse import bacc

    f32 = mybir.dt.float32
    bf16 = mybir.dt.bfloat16
    fp8 = mybir.dt.float8e4
    AF = mybir.ActivationFunctionType
    ALU = mybir.AluOpType
    DR = mybir.MatmulPerfMode.DoubleRow

    nc = bacc.Bacc("TRN2", target_bir_lowering=False, debug=False,
                   num_devices=N_CORES)

    # all pre-swizzled on host: partition dim first, contiguous per partition
    cax_d = nc.dram_tensor("cax8", [P, 2 * KX, Lc], fp8, kind="ExternalInput").ap()
    wvo_d = nc.dram_tensor("wvo8", [P, KX, C], fp8, kind="ExternalInput").ap()
    bvo_d = nc.dram_tensor("bvo32", [P, C], f32, kind="ExternalInput").ap()
    x8_d = nc.dram_tensor("x8", [P, N_HT * KC, HT], fp8, kind="ExternalInput").ap()
    xT_d = nc.dram_tensor("xT16", [P, HW // P, C], bf16, kind="ExternalInput").ap()
    wkbq_d = nc.dram_tensor("wkbq8", [P, KX], fp8, kind="ExternalInput").ap()
    y_d = nc.dram_tensor("yT", [P, HW // P, C], bf16, kind="ExternalOutput").ap()

    with tile.TileContext(nc) as tc:
        with (
            tc.tile_pool(name="const", bufs=1) as const,
            tc.tile_pool(name="work", bufs=3) as work,
            tc.tile_pool(name="yout", bufs=4) as yout,
            tc.tile_pool(name="small", bufs=3) as small,
            # PSUM budget (8 banks): sps 2x2 + mmy 3 + st 1
            tc.tile_pool(name="psum_s", bufs=2, space="PSUM") as psum_s,
            tc.tile_pool(name="psum_y", bufs=3, space="PSUM") as psum_y,
            tc.tile_pool(name="psum_st", bufs=1, space="PSUM") as psum_st,
        ):
            # ---------------- DMAs (ordered by when the PE needs them) -----
            # caxp[u] = [ctx_pair_u | A_pair_u], one tile+DMA per pair-block
            # so G's u-loop starts as soon as the first 256KB lands
            caxp = []
            for u in range(KX // 2):
                t = const.tile([P, 4, Lc], fp8, name=f"cax{u}", tag=f"cax{u}")
                nc.sync.dma_start(out=t, in_=cax_d[:, 4 * u:4 * u + 4, :])
                caxp.append(t)
            # x8 in three pieces: tiles 0-1 land early so sim0 starts ASAP
            x8s = []
            for q, (lo, hi) in enumerate([(0, 2), (2, 4), (4, 8)]):
                t = const.tile([P, (hi - lo) * KC, HT], fp8,
                               name=f"x8_{q}", tag=f"x8_{q}")
                x8s.append((lo, t))
            nc.sync.dma_start(out=x8s[0][1], in_=x8_d[:, 0:2 * KC, :])
            wvo = const.tile([P, KX, C], fp8, name="wvo", tag="wvo")
            nc.sync.dma_start(out=wvo, in_=wvo_d)
            if with_bvo:
                bvo = const.tile([P, C], f32, name="bvo", tag="bvo")
                nc.sync.dma_start(out=bvo, in_=bvo_d)
            if with_bq:
                wkbq = const.tile([P, KX], fp8, name="wkbq", tag="wkbq")
                nc.sync.dma_start(out=wkbq, in_=wkbq_d)
            xTt = []
            for q in range(4):
                t = const.tile([P, 8, C], bf16, name=f"xT{q}", tag=f"xT{q}")
                xTt.append(t)
            nc.sync.dma_start(out=xTt[0], in_=xT_d[:, 0:8, :])
            nc.sync.dma_start(out=x8s[1][1], in_=x8_d[:, 2 * KC:4 * KC, :])
            nc.sync.dma_start(out=xTt[1], in_=xT_d[:, 8:16, :])
            nc.sync.dma_start(out=x8s[2][1], in_=x8_d[:, 4 * KC:8 * KC, :])
            nc.sync.dma_start(out=xTt[2], in_=xT_d[:, 16:24, :])
            nc.sync.dma_start(out=xTt[3], in_=xT_d[:, 24:32, :])

            def x8_tile(h):
                for lo, t in reversed(x8s):
                    if h >= lo:
                        return t[:, (h - lo) * KC:(h - lo + 1) * KC, :]
                raise AssertionError

            # PE warm-up: dummy matmuls fill the initial DMA wait so the HAM
            # clock gate opens (1.2 -> 2.4 GHz) before real work arrives
            warm_sb = const.tile([P, HT], bf16, name="warm_sb", tag="warm")
            nc.vector.memset(warm_sb, 0.0)

            # ones (colsum moving operand) carry the vW storage scale so the
            # reciprocal unscales ps_y for free; 16-byte stride pad
            ones2 = const.tile([P, 2, 16], fp8, name="ones2", tag="ones2")
            nc.vector.memset(ones2, WSO)

            def keep_warm(n, name):
                for w in range(n):
                    ps_w = psum_y.tile([P, HT], f32, tag="mmy", name=f"{name}{w}")
                    nc.tensor.matmul(ps_w, warm_sb[:, :P], warm_sb,
                                     start=True, stop=True)

            keep_warm(N_WARM, "ps_warm")

            # ---------------- phase A --------------------------------------
            # G [128(c'), KC, Lc] = A ctx^T   (evictions split ACT/DVE).
            # mg pairs run u-outer so their matmuls stream with the cax
            # pair-block DMAs instead of stalling at u=2 until all 768KB land.
            G_8 = const.tile([P, KC, Lc], fp8, name="G_8", tag="G")
            for mgp in range(KC // 2):
                ps2 = [psum_y.tile([P, Lc], f32, tag="mmy",
                                   name=f"ps_g{2 * mgp + i}") for i in range(2)]
                for u in range(KX // 2):
                    for i in range(2):
                        mg = 2 * mgp + i
                        nc.tensor.matmul(ps2[i],
                                         caxp[u][:, 2:4, mg * P:(mg + 1) * P],
                                         caxp[u][:, 0:2, :],
                                         start=(u == 0),
                                         stop=(u == KX // 2 - 1),
                                         perf_mode=DR)
                nc.scalar.activation(G_8[:, 2 * mgp, :], ps2[0], AF.Copy,
                                     scale=1.0 / WSA)
                nc.vector.tensor_scalar_mul(G_8[:, 2 * mgp + 1, :], ps2[1],
                                            1.0 / WSA)

            # bqk_s [128(lc), KL] = SCALE * (Wk bq)^T ctx^T  (skipped if bq==0)
            if with_bq:
                bqk_s = const.tile([P, KL], f32, name="bqk_s", tag="bqk")
                for ml in range(KL):
                    ps = psum_st.tile([P, 16], f32, tag="st", name=f"ps_bq{ml}")
                    for u in range(KX // 2):
                        nc.tensor.matmul(ps[:, 0:1],
                                         caxp[u][:, 0:2, ml * P:(ml + 1) * P],
                                         wkbq[:, 2 * u:2 * u + 2],
                                         start=(u == 0), stop=(u == KX // 2 - 1),
                                         perf_mode=DR)
                    nc.scalar.activation(bqk_s[:, ml:ml + 1], ps[:, 0:1],
                                         AF.Identity, scale=SCALE / WSQ)

            # sim + exp for one hw tile.  One ACTIVATE per 2-bank psum pair
            # (exp cost is (N+352)/1.2 ns, so batching halves the overhead).
            def emit_sim(h):
                x_8 = x8_tile(h)
                eT = work.tile([P, KL, HT], fp8, tag="eT", name=f"eT_{h}")
                for half in range(2):
                    sps = psum_s.tile([P, 2, HT], f32, tag="sps",
                                      name=f"sps_{h}_{half}")
                    for mlh in range(2):
                        ml = 2 * half + mlh
                        for u in range(KC // 2):
                            nc.tensor.matmul(sps[:, mlh, :],
                                             G_8[:, 2 * u:2 * u + 2,
                                                 ml * P:(ml + 1) * P],
                                             x_8[:, 2 * u:2 * u + 2, :],
                                             start=(u == 0),
                                             stop=(u == KC // 2 - 1),
                                             perf_mode=DR)
                    if with_bq:
                        for mlh in range(2):
                            ml = 2 * half + mlh
                            nc.scalar.activation(eT[:, ml, :], sps[:, mlh, :],
                                                 AF.Exp, scale=SCALE,
                                                 bias=bqk_s[:, ml:ml + 1])
                    else:
                        nc.scalar.activation(eT[:, 2 * half:2 * half + 2, :],
                                             sps, AF.Exp, scale=SCALE)
                return eT

            eT0 = emit_sim(0)

            # vW [128(lc), KL, C] = ctx Wvo + bvo, stored at WSO scale
            vW_8 = const.tile([P, KL, C], fp8, name="vW_8", tag="vW")
            for ml in range(KL):
                ps = psum_y.tile([P, C], f32, tag="mmy", name=f"ps_vw{ml}")
                for u in range(KX // 2):
                    nc.tensor.matmul(ps,
                                     caxp[u][:, 0:2, ml * P:(ml + 1) * P],
                                     wvo[:, 2 * u:2 * u + 2, :],
                                     start=(u == 0), stop=(u == KX // 2 - 1),
                                     perf_mode=DR)
                if with_bvo:
                    nc.vector.scalar_tensor_tensor(
                        out=vW_8[:, ml, :], in0=ps, scalar=WSO / WSV, in1=bvo,
                        op0=ALU.mult, op1=ALU.add)
                else:
                    nc.vector.tensor_scalar_mul(vW_8[:, ml, :], ps, WSO / WSV)

            # ---------------- phase B: stream over hw tiles ----------------
            def emit_yT(h, eT, last=False):
                # yT [hw, c] = (eT^T vW) / colsum + x^T.  eT chunk is the
                # stationary for BOTH the colsum matvec (N=1) and the
                # attn@V matmul (N=512), sharing weight loads.  Both colsum
                # matmuls run BEFORE the two attn matmuls of a chunk so the
                # attn LDWEIGHTS hide under real matmul streaming, and the
                # reciprocal is split in two so evictions of chunks 0/1
                # overlap the matmuls of chunks 2/3.
                xT = xTt[h // 2]
                xo = (h % 2) * NCH
                y_sb = yout.tile([P, NCH, C], bf16, tag="y", name=f"y_{h}")
                ps_ys, sts = [], []

                def evict(ch, rec, rec_col):
                    # y = ps * (1/colsum)[per-partition] + xT in one op
                    nc.vector.scalar_tensor_tensor(
                        out=y_sb[:, ch, :], in0=ps_ys[ch],
                        scalar=rec[:, rec_col:rec_col + 1],
                        in1=xT[:, xo + ch, :],
                        op0=ALU.mult, op1=ALU.add)

                for half in range(2):
                    ps_st = psum_st.tile([P, 16], f32, tag="st",
                                         name=f"st_{h}_{half}")
                    sts.append(ps_st)
                    rec = small.tile([P, 16], f32, tag="rec",
                                     name=f"rec_{h}_{half}")
                    # per chunk: colsum pair then attn pair (the tiny
                    # colsum LDWs draft behind the 512-wide attn matmuls).
                    # The reciprocal is emitted between chunk1's colsum and
                    # attn pairs: its input is complete there, so evictions
                    # overlap chunk1's attn matmuls.
                    for chh in range(2):
                        ch = 2 * half + chh
                        cs = slice(ch * P, (ch + 1) * P)
                        ps_y = psum_y.tile([P, C], f32, tag="mmy",
                                           name=f"ps_y_{h}_{ch}")
                        ps_ys.append(ps_y)
                        for u in range(KL // 2):
                            nc.tensor.matmul(ps_st[:, chh:chh + 1],
                                             eT[:, 2 * u:2 * u + 2, cs],
                                             ones2[:, :, 0:1],
                                             start=(u == 0),
                                             stop=(u == KL // 2 - 1),
                                             perf_mode=DR)
                        if chh == 1:
                            nc.vector.reciprocal_approx_fast(
                                out=rec[:, 0:2], in_=ps_st[:, 0:2])
                        for u in range(KL // 2):
                            nc.tensor.matmul(ps_y,
                                             eT[:, 2 * u:2 * u + 2, cs],
                                             vW_8[:, 2 * u:2 * u + 2, :],
                                             start=(u == 0),
                                             stop=(u == KL // 2 - 1),
                                             perf_mode=DR)
                    if last:
                        # final tile: odd chunk evicts via ACT copy + DVE
                        # add so the drain runs on two engines; store each
                        # chunk as soon as it's evicted
                        evict(2 * half, rec, 0)
                        nc.sync.dma_start(
                            out=y_d[:, h * NCH + 2 * half:
                                    h * NCH + 2 * half + 1, :],
                            in_=y_sb[:, 2 * half:2 * half + 1, :])
                        ym = small.tile([P, C], bf16, tag="ymt",
                                        name=f"ymt_{h}_{half}")
                        nc.scalar.activation(ym, ps_ys[2 * half + 1],
                                             AF.Copy, scale=rec[:, 1:2])
                        nc.vector.tensor_add(
                            out=y_sb[:, 2 * half + 1, :], in0=ym,
                            in1=xT[:, xo + 2 * half + 1, :])
                        nc.sync.dma_start(
                            out=y_d[:, h * NCH + 2 * half + 1:
                                    h * NCH + 2 * half + 2, :],
                            in_=y_sb[:, 2 * half + 1:2 * half + 2, :])
                    else:
                        evict(2 * half, rec, 0)
                        evict(2 * half + 1, rec, 1)
                        if half == 1:
                            nc.sync.dma_start(
                                out=y_d[:, h * NCH:(h + 1) * NCH, :],
                                in_=y_sb)

            prev = (0, eT0)
            for h in range(1, N_HT):
                eT = emit_sim(h)
                # attn@V runs one tile behind (eT fully evicted by then)
                emit_yT(*prev)
                prev = (h, eT)

            emit_yT(*prev, last=True)

    nc.compile()
    return nc


def _get_compiled(with_bq=True, with_bvo=True):
    key = ("nc", with_bq, with_bvo)
    if key not in _cache:
        _cache[key] = _build_nc(with_bq, with_bvo)
    return _cache[key]


def _make_in_maps(x, context, Wq, bq, Wk, bk, Wv, bv, Wo, bo):
    x = np.asarray(x, dtype=np.float32)
    context = np.asarray(context, dtype=np.float32)
    Wq = np.asarray(Wq, np.float32)
    Wk = np.asarray(Wk, np.float32)
    Wv = np.asarray(Wv, np.float32)
    Wo = np.asarray(Wo, np.float32)
    bq = np.asarray(bq, np.float32)
    bv = np.asarray(bv, np.float32)
    bo = np.asarray(bo, np.float32)

    # exact host folds (see module docstring)
    A = Wq @ Wk.T                      # [C, CTX]
    Wvo = Wv @ Wo                      # [CTX, C]
    bvo = Wo.T @ bv + bo               # [C]
    wkbq = Wk @ bq                     # [CTX]

    def swz(M, k):
        # [k*P, n] row-major -> [P, k, n] with row r = k_idx*P + p
        n = M.shape[-1]
        return np.ascontiguousarray(M.reshape(k, P, n).transpose(1, 0, 2))

    common = {
        "wvo8": swz(Wvo * WSV, KX).astype(NP_FP8),
        "bvo32": np.ascontiguousarray(
            np.broadcast_to(bvo * WSO, (P, C))).astype(np.float32),
        "wkbq8": np.ascontiguousarray(
            (wkbq * WSQ).reshape(KX, P).T).astype(NP_FP8),
    }
    A_sw = swz(A.T * WSA, KX)          # [P, KX, C] fp8-ready

    in_maps = []
    for b in range(B):
        m = dict(common)
        xb = x[b].reshape(C, HW)
        ctx_sw = swz(context[b].T, KX)
        # interleaved pair-blocks: [ctx_u | A_u] for u in 0..2
        parts = []
        for u in range(KX // 2):
            parts.append(ctx_sw[:, 2 * u:2 * u + 2, :])
            parts.append(A_sw[:, 2 * u:2 * u + 2, :])
        m["cax8"] = np.ascontiguousarray(
            np.concatenate(parts, axis=1)).astype(NP_FP8)
        m["x8"] = np.ascontiguousarray(
            xb.reshape(KC, P, N_HT, HT).transpose(1, 2, 0, 3)
            .reshape(P, N_HT * KC, HT)).astype(NP_FP8)
        m["xT16"] = np.ascontiguousarray(
            xb.T.reshape(HW // P, P, C).transpose(1, 0, 2)).astype(NP_BF16)
        in_maps.append(m)
    return in_maps


def _run(in_maps, trace=False, with_bq=True, with_bvo=True):
    from concourse.bass_utils import run_bass_kernel_spmd
    nc = _get_compiled(with_bq, with_bvo)
    return run_bass_kernel_spmd(nc, in_maps, core_ids=list(range(N_CORES)),
                                trace=trace)


def _unswizzle_y(yr):
    # [P, HW//P, C] -> [C, H, W]
    yT = np.asarray(yr, dtype=np.float32).transpose(1, 0, 2).reshape(HW, C)
    return yT.T.reshape(C, HH, WW)


def kernel(x, context, Wq, bq, Wk, bk, Wv, bv, Wo, bo):
    in_maps = _make_in_maps(x, context, Wq, bq, Wk, bk, Wv, bv, Wo, bo)
    with_bq = bool(np.any(np.asarray(bq)))
    with_bvo = bool(np.any(np.asarray(bv))) or bool(np.any(np.asarray(bo)))
    res = _run(in_maps, trace=False, with_bq=with_bq, with_bvo=with_bvo)
    return np.stack([_unswizzle_y(res.results[b]["yT"]) for b in range(B)])
